# revision 35
# baseline (speedup 1.0000x reference)
"""Bilateral filter (joint/cross, 21-channel unaries, 3-channel guide) on 8 Trainium2 cores.

out[b,i,c,h,w] = sum_k wk * exp(-2*(I[b,i,p+dk]-I[b,i,p])^2) * Q[b,c,p+dk] / norm
(5x5 neighborhood minus center, zero padding, theta_alpha=1.5, theta_beta=0.5)

Sharding: pure data parallel over (batch, H-half) -> 8 shards, each core gets a
halo'd (132-row) padded shard and produces a (3,21,128,256) output block.

Per-core compute:
  - g[a][b-plane] = exp(-2*d^2 + ln(wk)) built with DVE subs + ACT Square/Exp.
  - 24 per-pixel products g (*) Q-window run on DVE + GpSimd (static split).
  - the 24-term accumulation runs on the PE array as identity-stationary
    float32r matmuls accumulating into PSUM (1 cycle/row at N=256).
  - normalization (reciprocal + multiply) reads PSUM directly; DMA out.
"""

import os
import sys

import numpy as np

_REPO = "/opt/trn_rl_repo"
if os.path.isdir(_REPO) and _REPO not in sys.path:
    sys.path.insert(0, _REPO)

import concourse.bacc as bacc
import concourse.bass as bass
import concourse.mybir as mybir
import concourse.tile as tile
from concourse.bass_utils import run_bass_kernel_spmd

F32 = mybir.dt.float32
F32R = mybir.dt.float32r

KS = 5
PAD = 2
THETA_ALPHA = 1.5
THETA_BETA = 0.5
EXP_SCALE = -1.0 / (2.0 * THETA_BETA * THETA_BETA)  # -2.0

B, CIN, NC_CH, H, W = 4, 3, 21, 256, 256
HOUT = 128           # output rows per core
HIN = HOUT + 2 * PAD  # 132 input rows per core (halo)
WP = W + 2 * PAD      # 260 padded cols
N_CORES = 8

# spatial (domain) weights wk[a][b], a/b in 0..4; center (2,2) excluded
_WK = np.exp(
    -(
        (np.arange(5)[:, None] - 2) ** 2 + (np.arange(5)[None, :] - 2) ** 2
    ).astype(np.float64)
    / (2.0 * THETA_ALPHA**2)
)
_LNWK = np.log(_WK)

# All 25 (a, b) plane indices in emission order; center stays (zeroed plane).
_PLANES = [(a, b) for a in range(5) for b in range(5)]


_BATCH_SUBS = True
_NORM_ON_ACT = False
_SUBS_ENG = "vector"  # "vector" | "gpsimd"
_G_DBUF = 3  # how many of the g tags get bufs=2
_PP_BUFS = 3
_PS_BUFS = 6
_EXACT_NORM = False  # norm via exact f32 DVE reduce (not f32r PE accumulate)


def _gps_split(ci, a):
    """Which product instructions run on GpSimd (vs DVE)."""
    return (a + ci) % 3 == 2


def _overlap_view(t, n_shift, width, elem_offset=0):
    """AP view [128, n_shift, width] of tile `t` where element (p, s, w)
    reads t[p, elem_offset + w + s] (both free strides 1 -> overlapping
    windows)."""
    ap = t[:] if not isinstance(t, bass.AP) else t
    pairs = [list(p) for p in ap.ap]
    part = pairs[0]
    return bass.AP(
        ap.tensor, ap.offset + elem_offset, [part, [1, n_shift], [1, width]]
    )


_PROG_CACHE = {}


def _build_program():
    """Build (once) the single-core Bass/Tile program shared by all 8 cores."""
    if "nc" in _PROG_CACHE:
        return _PROG_CACHE["nc"]

    nc = bacc.Bacc("TRN2", target_bir_lowering=False, debug=False)
    I_d = nc.dram_tensor("I_in", (CIN, HIN, WP), F32, kind="ExternalInput")
    Q_d = nc.dram_tensor("Q_in", (NC_CH, HIN, WP), F32, kind="ExternalInput")
    E_d = nc.dram_tensor("EYE", (128, 128), F32, kind="ExternalInput")
    O_d = nc.dram_tensor("OUT", (CIN, NC_CH, HOUT, W), F32, kind="ExternalOutput")

    with tile.TileContext(nc) as tc:
        with (
            tc.tile_pool(name="qp", bufs=1) as qp,
            tc.tile_pool(name="ip", bufs=1 if _EXACT_NORM else 2) as ip,
            tc.tile_pool(name="gp", bufs=1) as gp,
            tc.tile_pool(name="gp2", bufs=2) as gp2,
            tc.tile_pool(name="pp", bufs=_PP_BUFS) as pp,
            tc.tile_pool(name="op", bufs=2 if _EXACT_NORM else 6) as op,
            tc.tile_pool(name="cp", bufs=1) as cp,
            tc.tile_pool(name="rp", bufs=2) as rp,
            tc.tile_pool(name="ps", bufs=_PS_BUFS, space="PSUM") as ps,
        ):
            eye = cp.tile([128, 128], F32, tag="eye")
            nc.sync.dma_start(eye[:], E_d[:, :])
            # PE f32r matmuls require operands produced as float32r; the
            # identity's 0/1 values are exact under any rounding.
            eye_r = cp.tile([128, 128], F32R, tag="eye_r")
            nc.vector.tensor_copy(eye_r[:], eye[:])


            # per-partition bias tiles holding ln(wk[a,b]) for the Exp stage
            bias_t = {}
            for a in range(5):
                for b in range(5):
                    if a == 2 and b == 2:
                        continue
                    t = cp.tile([128, 1], F32, tag=f"bias{a}_{b}")
                    nc.gpsimd.memset(t[:], float(_LNWK[a, b]))
                    bias_t[(a, b)] = t

            # Q row-windows, one c-batched tile per vertical offset a:
            # qa[a] is [128, 21, 260] holding rows a..a+127 of every channel.
            qa = {}
            for a in range(5):
                t = qp.tile([128, NC_CH, WP], F32, tag=f"qa{a}")
                nc.sync.dma_start(t[:], Q_d[:, a : a + 128, :].transpose([1, 0, 2]))
                qa[a] = t

            for i in range(CIN):
                i0 = ip.tile([128, WP], F32, tag="i0")
                nc.sync.dma_start(i0[:], I_d[i, 2:130, :])
                ia = {}
                for a in range(5):
                    if a == 2:
                        ia[a] = i0
                        continue
                    t = ip.tile([128, WP], F32, tag=f"ia{a}")
                    nc.sync.dma_start(t[:], I_d[i, a : a + 128, :])
                    ia[a] = t

                # g[a]: [128, 5(b), 256] appearance*domain weights
                g = {}
                sub_eng = nc.gpsimd if _SUBS_ENG == "gpsimd" else nc.vector
                for a in range(5):
                    pool = gp2 if a < _G_DBUF else gp
                    gb = pool.tile([128, 5, W], F32, tag=f"g{a}")
                    if _BATCH_SUBS:
                        iav = _overlap_view(ia[a], 5, W)
                        i0b = i0[:, None, 2 : 2 + W].broadcast_to([128, 5, W])
                        sub_eng.tensor_sub(gb[:], iav, i0b)
                    else:
                        for b in range(5):
                            sub_eng.tensor_sub(
                                gb[:, b, :], ia[a][:, b : b + W], i0[:, 2 : 2 + W]
                            )
                    nc.scalar.activation(
                        gb[:], gb[:], mybir.ActivationFunctionType.Square
                    )
                    for b in range(5):
                        if a == 2 and b == 2:
                            continue
                        nc.scalar.activation(
                            gb[:, b, :],
                            gb[:, b, :],
                            mybir.ActivationFunctionType.Exp,
                            bias=bias_t[(a, b)][:],
                            scale=EXP_SCALE,
                        )
                    g[a] = gb
                # kill the (excluded) center tap
                nc.gpsimd.memset(g[2][:, 2, :], 0.0)

                # ---- norm (ci == -1) and the 21 unary channels.
                # Per (a): products g (*) Q-window -> f32r planes; PE
                # identity-matmuls accumulate the 25 planes into PSUM.
                # The per-channel tail (recip / normalize+store) is emitted one
                # iteration LATE so it never head-of-line blocks the DVE queue
                # behind the PE accumulation group it depends on.
                recip = None
                pending = None  # (ci, acc) awaiting its tail ops

                def _flush_tail():
                    nonlocal recip, pending
                    if pending is None:
                        return
                    pci, pacc = pending
                    pending = None
                    if pci < 0:
                        recip = rp.tile([128, W], F32, tag="recip")
                        rscratch = rp.tile([128, W], F32, tag="rscratch")
                        nc.vector.reciprocal_approx_accurate(
                            recip[:], pacc[:], rscratch[:]
                        )
                    else:
                        ob = op.tile([128, W], F32, tag="ob")
                        nc.vector.tensor_mul(ob[:], pacc[:], recip[:])
                        nc.sync.dma_start(O_d[i, pci, :, :], ob[:])

                if _EXACT_NORM:
                    # norm = sum over all 25 (a,b) planes of g, exact f32:
                    # per-a reduce over b (strided view), then chain-add.
                    ra = rp.tile([128, W], F32, tag="nra")
                    rb = rp.tile([128, W], F32, tag="nrb")
                    for a in range(5):
                        gap = g[a][:]
                        part = [list(p) for p in gap.ap][0]
                        bview = bass.AP(
                            gap.tensor, gap.offset, [part, [1, W], [W, 5]]
                        )
                        dst = ra if a == 0 else rb
                        nc.vector.tensor_reduce(
                            dst[:], bview, axis=mybir.AxisListType.X,
                            op=mybir.AluOpType.add,
                        )
                        if a > 0:
                            nc.vector.tensor_add(ra[:], ra[:], rb[:])
                    recip = rp.tile([128, W], F32, tag="recip")
                    rscratch = rp.tile([128, W], F32, tag="rscratch")
                    nc.vector.reciprocal_approx_accurate(
                        recip[:], ra[:], rscratch[:]
                    )

                ci_list = (
                    list(range(NC_CH)) if _EXACT_NORM
                    else [-1] + list(range(NC_CH))
                )
                for ci in ci_list:
                    acc = ps.tile([128, W], F32, tag="acc")
                    idx = 0
                    for a in range(5):
                        use_gps = ci >= 0 and _gps_split(ci, a)
                        pb = pp.tile(
                            [128, 5, W], F32R, tag="pbg" if use_gps else "pbv"
                        )
                        if ci < 0:
                            # norm channel: planes are just g itself
                            # (f32r-rounding copy).
                            if _NORM_ON_ACT:
                                nc.scalar.copy(pb[:], g[a][:])
                            else:
                                nc.vector.tensor_copy(pb[:], g[a][:])
                        else:
                            qv = _overlap_view(qa[a], 5, W, elem_offset=ci * WP)
                            eng = nc.gpsimd if use_gps else nc.vector
                            eng.tensor_mul(pb[:], g[a][:], qv)
                        if a == 2:
                            _flush_tail()
                        for b in range(5):
                            if a == 2 and b == 2:
                                continue  # center plane is identically zero
                            nc.tensor.matmul(
                                acc[:],
                                eye_r[:],
                                pb[:, b, :],
                                start=(idx == 0),
                                stop=(idx == 23),
                            )
                            idx += 1
                    pending = (ci, acc)
                _flush_tail()

    nc.compile()
    _PROG_CACHE["nc"] = nc
    return nc


def _make_in_maps(Q, I):
    Q = np.ascontiguousarray(np.asarray(Q, dtype=np.float32))
    I = np.ascontiguousarray(np.asarray(I, dtype=np.float32))
    Ip = np.zeros((B, CIN, H + 2 * PAD, WP), np.float32)
    Ip[:, :, PAD : PAD + H, PAD : PAD + W] = I
    Qp = np.zeros((B, NC_CH, H + 2 * PAD, WP), np.float32)
    Qp[:, :, PAD : PAD + H, PAD : PAD + W] = Q
    eye = np.ascontiguousarray(np.eye(128, dtype=np.float32))
    in_maps = []
    for core in range(N_CORES):
        b, half = divmod(core, 2)
        h0 = half * HOUT
        in_maps.append(
            {
                "I_in": np.ascontiguousarray(Ip[b, :, h0 : h0 + HIN, :]),
                "Q_in": np.ascontiguousarray(Qp[b, :, h0 : h0 + HIN, :]),
                "EYE": eye,
            }
        )
    return in_maps


def _assemble(results):
    out = np.zeros((B, CIN, NC_CH, H, W), np.float32)
    for core in range(N_CORES):
        b, half = divmod(core, 2)
        h0 = half * HOUT
        out[b, :, :, h0 : h0 + HOUT, :] = results[core]["OUT"]
    return out


def kernel(Q: np.ndarray, I: np.ndarray) -> np.ndarray:
    nc = _build_program()
    in_maps = _make_in_maps(Q, I)
    res = run_bass_kernel_spmd(nc, in_maps, core_ids=list(range(N_CORES)))
    return _assemble(res.results)



# revision 38
# speedup vs baseline: 1.0282x; 1.0282x over previous
"""Bilateral filter (joint/cross, 21-channel unaries, 3-channel guide) on 8 Trainium2 cores.

out[b,i,c,h,w] = sum_k wk * exp(-2*(I[b,i,p+dk]-I[b,i,p])^2) * Q[b,c,p+dk] / norm
(5x5 neighborhood minus center, zero padding, theta_alpha=1.5, theta_beta=0.5)

Sharding: pure data parallel over (batch, H-half) -> 8 shards, each core gets a
halo'd (132-row) padded shard and produces a (3,21,128,256) output block.

Per-core compute:
  - g[a][b-plane] = exp(-2*d^2 + ln(wk)) built with DVE subs + ACT Square/Exp.
  - 24 per-pixel products g (*) Q-window run on DVE + GpSimd (static split).
  - the 24-term accumulation runs on the PE array as identity-stationary
    float32r matmuls accumulating into PSUM (1 cycle/row at N=256).
  - normalization (reciprocal + multiply) reads PSUM directly; DMA out.
"""

import os
import sys

import numpy as np

_REPO = "/opt/trn_rl_repo"
if os.path.isdir(_REPO) and _REPO not in sys.path:
    sys.path.insert(0, _REPO)

import concourse.bacc as bacc
import concourse.bass as bass
import concourse.mybir as mybir
import concourse.tile as tile
from concourse.bass_utils import run_bass_kernel_spmd

F32 = mybir.dt.float32
F32R = mybir.dt.float32r

KS = 5
PAD = 2
THETA_ALPHA = 1.5
THETA_BETA = 0.5
EXP_SCALE = -1.0 / (2.0 * THETA_BETA * THETA_BETA)  # -2.0

B, CIN, NC_CH, H, W = 4, 3, 21, 256, 256
HOUT = 128           # output rows per core
HIN = HOUT + 2 * PAD  # 132 input rows per core (halo)
WP = W + 2 * PAD      # 260 padded cols
N_CORES = 8

# spatial (domain) weights wk[a][b], a/b in 0..4; center (2,2) excluded
_WK = np.exp(
    -(
        (np.arange(5)[:, None] - 2) ** 2 + (np.arange(5)[None, :] - 2) ** 2
    ).astype(np.float64)
    / (2.0 * THETA_ALPHA**2)
)
_LNWK = np.log(_WK)

# All 25 (a, b) plane indices in emission order; center stays (zeroed plane).
_PLANES = [(a, b) for a in range(5) for b in range(5)]


_BATCH_SUBS = True
_NORM_ON_ACT = False
_SUBS_ENG = "vector"  # "vector" | "gpsimd"
_G_DBUF = 3  # how many of the g tags get bufs=2
_PP_BUFS = 3
_PS_BUFS = 6
_EXACT_NORM = False  # norm via exact f32 DVE reduce (not f32r PE accumulate)
# Normalize tail: "vector" = DVE mult straight from PSUM;
# "act_gps" = ACT evacuates PSUM -> SBUF, GpSimd does the multiply
# (keeps the whole per-channel tail off the bottleneck DVE engine).
_NORM_MULT = "act_gps"


def _gps_split(ci, a):
    """Which product instructions run on GpSimd (vs DVE)."""
    return (a + ci) % 3 == 2


def _overlap_view(t, n_shift, width, elem_offset=0):
    """AP view [128, n_shift, width] of tile `t` where element (p, s, w)
    reads t[p, elem_offset + w + s] (both free strides 1 -> overlapping
    windows)."""
    ap = t[:] if not isinstance(t, bass.AP) else t
    pairs = [list(p) for p in ap.ap]
    part = pairs[0]
    return bass.AP(
        ap.tensor, ap.offset + elem_offset, [part, [1, n_shift], [1, width]]
    )


_PROG_CACHE = {}


def _build_program():
    """Build (once) the single-core Bass/Tile program shared by all 8 cores."""
    if "nc" in _PROG_CACHE:
        return _PROG_CACHE["nc"]

    nc = bacc.Bacc("TRN2", target_bir_lowering=False, debug=False)
    I_d = nc.dram_tensor("I_in", (CIN, HIN, WP), F32, kind="ExternalInput")
    Q_d = nc.dram_tensor("Q_in", (NC_CH, HIN, WP), F32, kind="ExternalInput")
    E_d = nc.dram_tensor("EYE", (128, 128), F32, kind="ExternalInput")
    O_d = nc.dram_tensor("OUT", (CIN, NC_CH, HOUT, W), F32, kind="ExternalOutput")

    with tile.TileContext(nc) as tc:
        with (
            tc.tile_pool(name="qp", bufs=1) as qp,
            tc.tile_pool(name="ip", bufs=1 if _EXACT_NORM else 2) as ip,
            tc.tile_pool(name="gp", bufs=1) as gp,
            tc.tile_pool(name="gp2", bufs=2) as gp2,
            tc.tile_pool(name="pp", bufs=_PP_BUFS) as pp,
            tc.tile_pool(
                name="op", bufs=3 if _NORM_MULT == "act_gps" else 6
            ) as op,
            tc.tile_pool(name="cp", bufs=1) as cp,
            tc.tile_pool(name="rp", bufs=2) as rp,
            tc.tile_pool(name="ps", bufs=_PS_BUFS, space="PSUM") as ps,
        ):
            eye = cp.tile([128, 128], F32, tag="eye")
            nc.sync.dma_start(eye[:], E_d[:, :])
            # PE f32r matmuls require operands produced as float32r; the
            # identity's 0/1 values are exact under any rounding.
            eye_r = cp.tile([128, 128], F32R, tag="eye_r")
            nc.vector.tensor_copy(eye_r[:], eye[:])


            # per-partition bias tiles holding ln(wk[a,b]) for the Exp stage
            bias_t = {}
            for a in range(5):
                for b in range(5):
                    if a == 2 and b == 2:
                        continue
                    t = cp.tile([128, 1], F32, tag=f"bias{a}_{b}")
                    nc.gpsimd.memset(t[:], float(_LNWK[a, b]))
                    bias_t[(a, b)] = t

            # Q row-windows, one c-batched tile per vertical offset a:
            # qa[a] is [128, 21, 260] holding rows a..a+127 of every channel.
            qa = {}
            for a in range(5):
                t = qp.tile([128, NC_CH, WP], F32, tag=f"qa{a}")
                nc.sync.dma_start(t[:], Q_d[:, a : a + 128, :].transpose([1, 0, 2]))
                qa[a] = t

            for i in range(CIN):
                i0 = ip.tile([128, WP], F32, tag="i0")
                nc.sync.dma_start(i0[:], I_d[i, 2:130, :])
                ia = {}
                for a in range(5):
                    if a == 2:
                        ia[a] = i0
                        continue
                    t = ip.tile([128, WP], F32, tag=f"ia{a}")
                    nc.sync.dma_start(t[:], I_d[i, a : a + 128, :])
                    ia[a] = t

                # g[a]: [128, 5(b), 256] appearance*domain weights
                g = {}
                sub_eng = nc.gpsimd if _SUBS_ENG == "gpsimd" else nc.vector
                for a in range(5):
                    pool = gp2 if a < _G_DBUF else gp
                    gb = pool.tile([128, 5, W], F32, tag=f"g{a}")
                    if _BATCH_SUBS:
                        iav = _overlap_view(ia[a], 5, W)
                        i0b = i0[:, None, 2 : 2 + W].broadcast_to([128, 5, W])
                        sub_eng.tensor_sub(gb[:], iav, i0b)
                    else:
                        for b in range(5):
                            sub_eng.tensor_sub(
                                gb[:, b, :], ia[a][:, b : b + W], i0[:, 2 : 2 + W]
                            )
                    nc.scalar.activation(
                        gb[:], gb[:], mybir.ActivationFunctionType.Square
                    )
                    for b in range(5):
                        if a == 2 and b == 2:
                            continue
                        nc.scalar.activation(
                            gb[:, b, :],
                            gb[:, b, :],
                            mybir.ActivationFunctionType.Exp,
                            bias=bias_t[(a, b)][:],
                            scale=EXP_SCALE,
                        )
                    g[a] = gb
                # kill the (excluded) center tap
                nc.gpsimd.memset(g[2][:, 2, :], 0.0)

                # ---- norm (ci == -1) and the 21 unary channels.
                # Per (a): products g (*) Q-window -> f32r planes; PE
                # identity-matmuls accumulate the 25 planes into PSUM.
                # The per-channel tail (recip / normalize+store) is emitted one
                # iteration LATE so it never head-of-line blocks the DVE queue
                # behind the PE accumulation group it depends on.
                recip = None
                pending = None  # (ci, acc) awaiting its tail ops

                def _flush_tail():
                    nonlocal recip, pending
                    if pending is None:
                        return
                    pci, pacc = pending
                    pending = None
                    if pci < 0:
                        recip = rp.tile([128, W], F32, tag="recip")
                        rscratch = rp.tile([128, W], F32, tag="rscratch")
                        nc.vector.reciprocal_approx_accurate(
                            recip[:], pacc[:], rscratch[:]
                        )
                    else:
                        ob = op.tile([128, W], F32, tag="ob")
                        if _NORM_MULT == "act_gps":
                            ob1 = op.tile([128, W], F32, tag="ob1")
                            nc.scalar.copy(ob1[:], pacc[:])
                            nc.gpsimd.tensor_mul(ob[:], ob1[:], recip[:])
                        else:
                            nc.vector.tensor_mul(ob[:], pacc[:], recip[:])
                        nc.sync.dma_start(O_d[i, pci, :, :], ob[:])

                if _EXACT_NORM:
                    # norm = sum over all 25 (a,b) planes of g, exact f32:
                    # per-a reduce over b (strided view), then chain-add.
                    ra = rp.tile([128, W], F32, tag="nra")
                    rb = rp.tile([128, W], F32, tag="nrb")
                    for a in range(5):
                        gap = g[a][:]
                        part = [list(p) for p in gap.ap][0]
                        bview = bass.AP(
                            gap.tensor, gap.offset, [part, [1, W], [W, 5]]
                        )
                        dst = ra if a == 0 else rb
                        nc.vector.tensor_reduce(
                            dst[:], bview, axis=mybir.AxisListType.X,
                            op=mybir.AluOpType.add,
                        )
                        if a > 0:
                            nc.vector.tensor_add(ra[:], ra[:], rb[:])
                    recip = rp.tile([128, W], F32, tag="recip")
                    rscratch = rp.tile([128, W], F32, tag="rscratch")
                    nc.vector.reciprocal_approx_accurate(
                        recip[:], ra[:], rscratch[:]
                    )

                ci_list = (
                    list(range(NC_CH)) if _EXACT_NORM
                    else [-1] + list(range(NC_CH))
                )
                for ci in ci_list:
                    acc = ps.tile([128, W], F32, tag="acc")
                    idx = 0
                    for a in range(5):
                        use_gps = ci >= 0 and _gps_split(ci, a)
                        pb = pp.tile(
                            [128, 5, W], F32R, tag="pbg" if use_gps else "pbv"
                        )
                        if ci < 0:
                            # norm channel: planes are just g itself
                            # (f32r-rounding copy).
                            if _NORM_ON_ACT:
                                nc.scalar.copy(pb[:], g[a][:])
                            else:
                                nc.vector.tensor_copy(pb[:], g[a][:])
                        else:
                            qv = _overlap_view(qa[a], 5, W, elem_offset=ci * WP)
                            eng = nc.gpsimd if use_gps else nc.vector
                            eng.tensor_mul(pb[:], g[a][:], qv)
                        if a == 2:
                            _flush_tail()
                        for b in range(5):
                            if a == 2 and b == 2:
                                continue  # center plane is identically zero
                            nc.tensor.matmul(
                                acc[:],
                                eye_r[:],
                                pb[:, b, :],
                                start=(idx == 0),
                                stop=(idx == 23),
                            )
                            idx += 1
                    pending = (ci, acc)
                _flush_tail()

    nc.compile()
    _PROG_CACHE["nc"] = nc
    return nc


def _make_in_maps(Q, I):
    Q = np.ascontiguousarray(np.asarray(Q, dtype=np.float32))
    I = np.ascontiguousarray(np.asarray(I, dtype=np.float32))
    Ip = np.zeros((B, CIN, H + 2 * PAD, WP), np.float32)
    Ip[:, :, PAD : PAD + H, PAD : PAD + W] = I
    Qp = np.zeros((B, NC_CH, H + 2 * PAD, WP), np.float32)
    Qp[:, :, PAD : PAD + H, PAD : PAD + W] = Q
    eye = np.ascontiguousarray(np.eye(128, dtype=np.float32))
    in_maps = []
    for core in range(N_CORES):
        b, half = divmod(core, 2)
        h0 = half * HOUT
        in_maps.append(
            {
                "I_in": np.ascontiguousarray(Ip[b, :, h0 : h0 + HIN, :]),
                "Q_in": np.ascontiguousarray(Qp[b, :, h0 : h0 + HIN, :]),
                "EYE": eye,
            }
        )
    return in_maps


def _assemble(results):
    out = np.zeros((B, CIN, NC_CH, H, W), np.float32)
    for core in range(N_CORES):
        b, half = divmod(core, 2)
        h0 = half * HOUT
        out[b, :, :, h0 : h0 + HOUT, :] = results[core]["OUT"]
    return out


def kernel(Q: np.ndarray, I: np.ndarray) -> np.ndarray:
    nc = _build_program()
    in_maps = _make_in_maps(Q, I)
    res = run_bass_kernel_spmd(nc, in_maps, core_ids=list(range(N_CORES)))
    return _assemble(res.results)



# revision 43
# speedup vs baseline: 1.0385x; 1.0100x over previous
"""Bilateral filter (joint/cross, 21-channel unaries, 3-channel guide) on 8 Trainium2 cores.

out[b,i,c,h,w] = sum_k wk * exp(-2*(I[b,i,p+dk]-I[b,i,p])^2) * Q[b,c,p+dk] / norm
(5x5 neighborhood minus center, zero padding, theta_alpha=1.5, theta_beta=0.5)

Sharding: pure data parallel over (batch, H-half) -> 8 shards, each core gets a
halo'd (132-row) padded shard and produces a (3,21,128,256) output block.

Per-core compute:
  - g[a][b-plane] = exp(-2*d^2 + ln(wk)) built with DVE subs + ACT Square/Exp.
  - 24 per-pixel products g (*) Q-window run on DVE + GpSimd (static split).
  - the 24-term accumulation runs on the PE array as identity-stationary
    float32r matmuls accumulating into PSUM (1 cycle/row at N=256).
  - normalization (reciprocal + multiply) reads PSUM directly; DMA out.
"""

import os
import sys

import numpy as np

_REPO = "/opt/trn_rl_repo"
if os.path.isdir(_REPO) and _REPO not in sys.path:
    sys.path.insert(0, _REPO)

import concourse.bacc as bacc
import concourse.bass as bass
import concourse.mybir as mybir
import concourse.tile as tile
from concourse.bass_utils import run_bass_kernel_spmd

F32 = mybir.dt.float32
F32R = mybir.dt.float32r

KS = 5
PAD = 2
THETA_ALPHA = 1.5
THETA_BETA = 0.5
EXP_SCALE = -1.0 / (2.0 * THETA_BETA * THETA_BETA)  # -2.0

B, CIN, NC_CH, H, W = 4, 3, 21, 256, 256
HOUT = 128           # output rows per core
HIN = HOUT + 2 * PAD  # 132 input rows per core (halo)
WP = W + 2 * PAD      # 260 padded cols
N_CORES = 8

# spatial (domain) weights wk[a][b], a/b in 0..4; center (2,2) excluded
_WK = np.exp(
    -(
        (np.arange(5)[:, None] - 2) ** 2 + (np.arange(5)[None, :] - 2) ** 2
    ).astype(np.float64)
    / (2.0 * THETA_ALPHA**2)
)
_LNWK = np.log(_WK)

# All 25 (a, b) plane indices in emission order; center stays (zeroed plane).
_PLANES = [(a, b) for a in range(5) for b in range(5)]


_BATCH_SUBS = True
_NORM_ON_ACT = False
_SUBS_ENG = "vector"  # "vector" | "gpsimd"
_G_DBUF = 3  # how many of the g tags get bufs=2
_PP_BUFS = 3
_PS_BUFS = 6
_EXACT_NORM = False  # norm via exact f32 DVE reduce (not f32r PE accumulate)
# Normalize tail: "vector" = DVE mult straight from PSUM;
# "act_gps" = ACT evacuates PSUM -> SBUF, GpSimd does the multiply
# (keeps the whole per-channel tail off the bottleneck DVE engine).
_NORM_MULT = "act_gps"


def _gps_split(ci, a):
    """Which product instructions run on GpSimd (vs DVE)."""
    return (a + ci) % 3 == 2


def _overlap_view(t, n_shift, width, elem_offset=0):
    """AP view [128, n_shift, width] of tile `t` where element (p, s, w)
    reads t[p, elem_offset + w + s] (both free strides 1 -> overlapping
    windows)."""
    ap = t[:] if not isinstance(t, bass.AP) else t
    pairs = [list(p) for p in ap.ap]
    part = pairs[0]
    return bass.AP(
        ap.tensor, ap.offset + elem_offset, [part, [1, n_shift], [1, width]]
    )


_PROG_CACHE = {}


def _build_program():
    """Build (once) the single-core Bass/Tile program shared by all 8 cores."""
    if "nc" in _PROG_CACHE:
        return _PROG_CACHE["nc"]

    nc = bacc.Bacc("TRN2", target_bir_lowering=False, debug=False)
    I_d = nc.dram_tensor("I_in", (CIN, HIN, WP), F32, kind="ExternalInput")
    Q_d = nc.dram_tensor("Q_in", (NC_CH, HIN, WP), F32, kind="ExternalInput")
    E_d = nc.dram_tensor("EYE", (128, 128), F32, kind="ExternalInput")
    O_d = nc.dram_tensor("OUT", (CIN, NC_CH, HOUT, W), F32, kind="ExternalOutput")

    with tile.TileContext(nc) as tc:
        with (
            tc.tile_pool(name="qp", bufs=1) as qp,
            tc.tile_pool(name="ip", bufs=1 if _EXACT_NORM else 2) as ip,
            tc.tile_pool(name="gp", bufs=1) as gp,
            tc.tile_pool(name="gp2", bufs=2) as gp2,
            tc.tile_pool(name="pp", bufs=_PP_BUFS) as pp,
            tc.tile_pool(
                name="op", bufs=3 if _NORM_MULT == "act_gps" else 6
            ) as op,
            tc.tile_pool(name="cp", bufs=1) as cp,
            tc.tile_pool(name="rp", bufs=2) as rp,
            tc.tile_pool(name="rp1", bufs=1) as rp1,
            tc.tile_pool(name="ps", bufs=_PS_BUFS, space="PSUM") as ps,
        ):
            eye = cp.tile([128, 128], F32, tag="eye")
            nc.sync.dma_start(eye[:], E_d[:, :])
            # PE f32r matmuls require operands produced as float32r; the
            # identity's 0/1 values are exact under any rounding. wk is
            # separable (wr[a]*wc[b]): wr rides the Exp bias, wc rides
            # scaled-identity stationaries (3 distinct values; wc[2]=1).
            eye_r = cp.tile([128, 128], F32R, tag="eye_r")
            nc.vector.tensor_copy(eye_r[:], eye[:])
            _wc = np.exp(-((np.arange(5) - 2.0) ** 2) / (2.0 * THETA_ALPHA**2))
            eye_b = {}
            for b in range(5):
                if b == 2:
                    eye_b[b] = eye_r
                elif (4 - b) in eye_b and abs(_wc[b] - _wc[4 - b]) < 1e-12:
                    eye_b[b] = eye_b[4 - b]
                else:
                    t = cp.tile([128, 128], F32R, tag=f"eye_b{b}")
                    nc.vector.tensor_scalar_mul(t[:], eye[:], float(_wc[b]))
                    eye_b[b] = t

            # per-partition bias tiles holding ln(wr[a]) for the Exp stage
            bias_t = {}
            for a in range(5):
                t = cp.tile([128, 1], F32, tag=f"bias{a}")
                nc.gpsimd.memset(t[:], float(np.log(_wc[a])))
                bias_t[a] = t

            # Q row-windows, one c-batched tile per vertical offset a:
            # qa[a] is [128, 21, 260] holding rows a..a+127 of every channel.
            qa = {}
            for a in range(5):
                t = qp.tile([128, NC_CH, WP], F32, tag=f"qa{a}")
                nc.sync.dma_start(t[:], Q_d[:, a : a + 128, :].transpose([1, 0, 2]))
                qa[a] = t

            for i in range(CIN):
                i0 = ip.tile([128, WP], F32, tag="i0")
                nc.sync.dma_start(i0[:], I_d[i, 2:130, :])
                ia = {}
                for a in range(5):
                    if a == 2:
                        ia[a] = i0
                        continue
                    t = ip.tile([128, WP], F32, tag=f"ia{a}")
                    nc.sync.dma_start(t[:], I_d[i, a : a + 128, :])
                    ia[a] = t

                # g[a]: [128, 5(b), 256] appearance*domain weights
                g = {}
                sub_eng = nc.gpsimd if _SUBS_ENG == "gpsimd" else nc.vector
                for a in range(5):
                    pool = gp2 if a < _G_DBUF else gp
                    gb = pool.tile([128, 5, W], F32, tag=f"g{a}")
                    if _BATCH_SUBS:
                        iav = _overlap_view(ia[a], 5, W)
                        i0b = i0[:, None, 2 : 2 + W].broadcast_to([128, 5, W])
                        sub_eng.tensor_sub(gb[:], iav, i0b)
                    else:
                        for b in range(5):
                            sub_eng.tensor_sub(
                                gb[:, b, :], ia[a][:, b : b + W], i0[:, 2 : 2 + W]
                            )
                    nc.scalar.activation(
                        gb[:], gb[:], mybir.ActivationFunctionType.Square
                    )
                    nc.scalar.activation(
                        gb[:],
                        gb[:],
                        mybir.ActivationFunctionType.Exp,
                        bias=bias_t[a][:],
                        scale=EXP_SCALE,
                    )
                    g[a] = gb
                # kill the (excluded) center tap
                nc.gpsimd.memset(g[2][:, 2, :], 0.0)

                # ---- norm (ci == -1) and the 21 unary channels.
                # Per (a): products g (*) Q-window -> f32r planes; PE
                # identity-matmuls accumulate the 25 planes into PSUM.
                # The per-channel tail (recip / normalize+store) is emitted one
                # iteration LATE so it never head-of-line blocks the DVE queue
                # behind the PE accumulation group it depends on.
                recip = None
                pending = None  # (ci, acc) awaiting its tail ops

                def _flush_tail():
                    nonlocal recip, pending
                    if pending is None:
                        return
                    pci, pacc = pending
                    pending = None
                    if pci < 0:
                        recip = rp.tile([128, W], F32, tag="recip")
                        rscratch = rp1.tile([128, W], F32, tag="rscratch")
                        nc.vector.reciprocal_approx_accurate(
                            recip[:], pacc[:], rscratch[:]
                        )
                    else:
                        ob = op.tile([128, W], F32, tag="ob")
                        if _NORM_MULT == "act_gps":
                            ob1 = op.tile([128, W], F32, tag="ob1")
                            nc.scalar.copy(ob1[:], pacc[:])
                            nc.gpsimd.tensor_mul(ob[:], ob1[:], recip[:])
                        else:
                            nc.vector.tensor_mul(ob[:], pacc[:], recip[:])
                        nc.sync.dma_start(O_d[i, pci, :, :], ob[:])

                if _EXACT_NORM:
                    # norm = sum over all 25 (a,b) planes of g, exact f32:
                    # per-a reduce over b (strided view), then chain-add.
                    ra = rp.tile([128, W], F32, tag="nra")
                    rb = rp.tile([128, W], F32, tag="nrb")
                    for a in range(5):
                        gap = g[a][:]
                        part = [list(p) for p in gap.ap][0]
                        bview = bass.AP(
                            gap.tensor, gap.offset, [part, [1, W], [W, 5]]
                        )
                        dst = ra if a == 0 else rb
                        nc.vector.tensor_reduce(
                            dst[:], bview, axis=mybir.AxisListType.X,
                            op=mybir.AluOpType.add,
                        )
                        if a > 0:
                            nc.vector.tensor_add(ra[:], ra[:], rb[:])
                    recip = rp.tile([128, W], F32, tag="recip")
                    rscratch = rp.tile([128, W], F32, tag="rscratch")
                    nc.vector.reciprocal_approx_accurate(
                        recip[:], ra[:], rscratch[:]
                    )

                ci_list = (
                    list(range(NC_CH)) if _EXACT_NORM
                    else [-1] + list(range(NC_CH))
                )
                for ci in ci_list:
                    acc = ps.tile([128, W], F32, tag="acc")
                    idx = 0
                    for a in range(5):
                        use_gps = ci >= 0 and _gps_split(ci, a)
                        pb = pp.tile(
                            [128, 5, W], F32R, tag="pbg" if use_gps else "pbv"
                        )
                        if ci < 0:
                            # norm channel: planes are just g itself
                            # (f32r-rounding copy).
                            if _NORM_ON_ACT:
                                nc.scalar.copy(pb[:], g[a][:])
                            else:
                                nc.vector.tensor_copy(pb[:], g[a][:])
                        else:
                            qv = _overlap_view(qa[a], 5, W, elem_offset=ci * WP)
                            eng = nc.gpsimd if use_gps else nc.vector
                            eng.tensor_mul(pb[:], g[a][:], qv)
                        if a == 2:
                            _flush_tail()
                        for b in range(5):
                            if a == 2 and b == 2:
                                continue  # center plane is identically zero
                            nc.tensor.matmul(
                                acc[:],
                                eye_b[b][:],
                                pb[:, b, :],
                                start=(idx == 0),
                                stop=(idx == 23),
                            )
                            idx += 1
                    pending = (ci, acc)
                _flush_tail()

    nc.compile()
    _PROG_CACHE["nc"] = nc
    return nc


def _make_in_maps(Q, I):
    Q = np.ascontiguousarray(np.asarray(Q, dtype=np.float32))
    I = np.ascontiguousarray(np.asarray(I, dtype=np.float32))
    Ip = np.zeros((B, CIN, H + 2 * PAD, WP), np.float32)
    Ip[:, :, PAD : PAD + H, PAD : PAD + W] = I
    Qp = np.zeros((B, NC_CH, H + 2 * PAD, WP), np.float32)
    Qp[:, :, PAD : PAD + H, PAD : PAD + W] = Q
    eye = np.ascontiguousarray(np.eye(128, dtype=np.float32))
    in_maps = []
    for core in range(N_CORES):
        b, half = divmod(core, 2)
        h0 = half * HOUT
        in_maps.append(
            {
                "I_in": np.ascontiguousarray(Ip[b, :, h0 : h0 + HIN, :]),
                "Q_in": np.ascontiguousarray(Qp[b, :, h0 : h0 + HIN, :]),
                "EYE": eye,
            }
        )
    return in_maps


def _assemble(results):
    out = np.zeros((B, CIN, NC_CH, H, W), np.float32)
    for core in range(N_CORES):
        b, half = divmod(core, 2)
        h0 = half * HOUT
        out[b, :, :, h0 : h0 + HOUT, :] = results[core]["OUT"]
    return out


def kernel(Q: np.ndarray, I: np.ndarray) -> np.ndarray:
    nc = _build_program()
    in_maps = _make_in_maps(Q, I)
    res = run_bass_kernel_spmd(nc, in_maps, core_ids=list(range(N_CORES)))
    return _assemble(res.results)

